# revision 1
# baseline (speedup 1.0000x reference)
"""Trainium2 Bass kernel for nn_KernelProjectionT2I.

Sharding: data-parallel over the caption axis (B_cap=48 -> 6 captions per
core on 8 cores). Each core holds the full image batch + conv weights and
computes the (B_img, 6) similarity columns for its captions; the host
concatenates the per-core columns.

Math (per caption q):
  cap0 = cap_embed[q, 0]                          (1024,)
  cap_repr = Wred @ cap0 + bred                   (256,)
  wdyn = softmax_K((Wproj @ cap_repr + bproj).reshape(1024, 3))
  Xconv[c, n] = w0[c] x[c, r-1] + w1[c] x[c, r] + w2[c] x[c, r+1]
  y = Wconv @ Xconv            (bias bconv dropped: softmax over regions is
                                shift-invariant, so pooled = pooled' + bconv)
  A = sum_r exp(y), Bsum = sum_r y exp(y)         (gated pool, per (b, d))
  img_vec = Bsum/A + bconv ;  sims[b, q] = <img_vec, cap0> / (|img_vec||cap0|)

Device layout: n = (b, r) on PSUM partitions for y (so region sums run on the
TensorEngine as 0/1-selector matmuls), channel c on SBUF partitions for the
depthwise stage (per-partition-scalar fused MACs). Big matmuls use float32r
(FP22 single pass).
"""

import numpy as np
from contextlib import ExitStack

import concourse.bass as bass
import concourse.tile as tile
from concourse import bacc, mybir
from concourse.bass_utils import run_bass_kernel_spmd

F32 = mybir.dt.float32
F32R = mybir.dt.float32r
AF = mybir.ActivationFunctionType
OP = mybir.AluOpType

N_CORES = 8
B, R, D = 48, 36, 1024
Q = 48
QL = Q // N_CORES          # 6 captions per core
DQ, DK, K = 256, 3072, 3
NB = B * R                 # 1728
NP = 1792                  # padded to 14 chunks of 128
NCH = NP // 128            # 14
# Xconv slabs, both b-aligned and 128-aligned (lcm(36,128)=1152)
SLABS = [(0, 32, 0, 9), (32, 16, 9, 5)]   # (b0, nb, nch0, n_nchunks)

LAST_EXEC_NS = None
_CACHE = {}
import os
STAGE = int(os.environ.get("KSTAGE", "9"))


def _build_nc():
    nc = bacc.Bacc(trn_type="TRN2", target_bir_lowering=False,
                   num_devices=N_CORES)
    x38_d = nc.dram_tensor("x38", [128, 8, B, 38], F32, kind="ExternalInput")
    wct_d = nc.dram_tensor("wct", [128, 8, D], F32R, kind="ExternalInput")
    wrt_d = nc.dram_tensor("wrt", [128, 8, DQ], F32, kind="ExternalInput")
    wpp_d = nc.dram_tensor("wpp", [128, 2, K, D], F32, kind="ExternalInput")
    bred_d = nc.dram_tensor("bred", [128, 2], F32, kind="ExternalInput")
    bpp_d = nc.dram_tensor("bpp", [128, 8, K], F32, kind="ExternalInput")
    sel_d = nc.dram_tensor("sel", [128, NCH, B], F32R, kind="ExternalInput")
    bcb_d = nc.dram_tensor("bcb", [B, D], F32, kind="ExternalInput")
    capt_d = nc.dram_tensor("capt", [128, 8, QL], F32, kind="ExternalInput")
    capb_d = nc.dram_tensor("capb", [QL, B, D], F32, kind="ExternalInput")
    out_d = nc.dram_tensor("out", [B, QL], F32, kind="ExternalOutput")

    with ExitStack() as ctx:
        tc = ctx.enter_context(tile.TileContext(nc))
        const = ctx.enter_context(tc.tile_pool(name="const", bufs=1))
        xcp = ctx.enter_context(tc.tile_pool(name="xcp", bufs=2))
        ep = ctx.enter_context(tc.tile_pool(name="ep", bufs=2))
        qv = ctx.enter_context(tc.tile_pool(name="qv", bufs=1))
        small = ctx.enter_context(tc.tile_pool(name="small", bufs=2))
        psy = ctx.enter_context(tc.tile_pool(name="psy", bufs=2, space="PSUM"))
        psA = ctx.enter_context(tc.tile_pool(name="psA", bufs=1, space="PSUM"))
        psB = ctx.enter_context(tc.tile_pool(name="psB", bufs=1, space="PSUM"))

        # ---- resident inputs ----
        capt_t = const.tile([128, 8, QL], F32)
        nc.sync.dma_start(out=capt_t, in_=capt_d.ap())
        bred_t = const.tile([128, 2], F32)
        nc.sync.dma_start(out=bred_t, in_=bred_d.ap())
        bpp_t = const.tile([128, 8, K], F32)
        nc.sync.dma_start(out=bpp_t, in_=bpp_d.ap())
        x38_t = const.tile([128, 8, B, 38], F32)
        nc.sync.dma_start(out=x38_t, in_=x38_d.ap())
        wct_t = const.tile([128, 8, D], F32R)
        nc.sync.dma_start(out=wct_t, in_=wct_d.ap())
        sel_t = const.tile([128, NCH, B], F32R)
        nc.sync.dma_start(out=sel_t, in_=sel_d.ap())
        bcb_t = const.tile([B, D], F32)
        nc.sync.dma_start(out=bcb_t, in_=bcb_d.ap())

        # MLP weights share the Xconv slab slots (used once, up front)
        wrt_t = xcp.tile([128, 8, DQ], F32, tag="xc")
        nc.sync.dma_start(out=wrt_t, in_=wrt_d.ap())
        wpp_t = xcp.tile([128, 2, K, D], F32, tag="xc")
        nc.sync.dma_start(out=wpp_t, in_=wpp_d.ap())

        out_sb = const.tile([B, QL], F32)
        nc.vector.memset(out_sb, 0.0)
        z64 = const.tile([128, 64], F32)
        nc.vector.memset(z64, 0.0)
        dot_t = const.tile([B, QL], F32)
        s2_t = const.tile([B, QL], F32)
        s2c_t = const.tile([B, QL], F32)

        # ---- caption MLP for all local captions (full fp32) ----
        repr_ps = psB.tile([128, 2, QL], F32, tag="B")
        for mc in range(2):
            for cc in range(8):
                nc.tensor.matmul(repr_ps[:, mc, :],
                                 lhsT=wrt_t[:, cc, mc * 128:(mc + 1) * 128],
                                 rhs=capt_t[:, cc, :],
                                 start=(cc == 0), stop=(cc == 7))
        repr_sb = small.tile([128, 2, QL], F32)
        for mc in range(2):
            nc.vector.tensor_scalar_add(repr_sb[:, mc, :], repr_ps[:, mc, :],
                                        bred_t[:, mc:mc + 1])

        L_ps = [psy.tile([128, 8, QL], F32, tag="y", name="L0"),
                psy.tile([128, 8, QL], F32, tag="y", name="L1"),
                psA.tile([128, 8, QL], F32, tag="A", name="L2")]
        for kk in range(K):
            for mc in range(8):
                nc.tensor.matmul(L_ps[kk][:, mc, :],
                                 lhsT=wpp_t[:, 0, kk, mc * 128:(mc + 1) * 128],
                                 rhs=repr_sb[:, 0, :], start=True, stop=False)
                nc.tensor.matmul(L_ps[kk][:, mc, :],
                                 lhsT=wpp_t[:, 1, kk, mc * 128:(mc + 1) * 128],
                                 rhs=repr_sb[:, 1, :], start=False, stop=True)

        # softmax over the K taps (no max-sub: |logits| ~ N(0,1))
        e_k = [small.tile([128, 8, QL], F32, name=f"ek{i}") for i in range(K)]
        for kk in range(K):
            for mc in range(8):
                nc.scalar.activation(e_k[kk][:, mc, :], L_ps[kk][:, mc, :],
                                     AF.Exp, bias=bpp_t[:, mc, kk:kk + 1])
        ssum = small.tile([128, 8, QL], F32)
        nc.vector.tensor_add(ssum, e_k[0], e_k[1])
        nc.vector.tensor_add(ssum, ssum, e_k[2])
        rinv = small.tile([128, 8, QL], F32)
        nc.vector.reciprocal(rinv, ssum)
        w_t = [const.tile([128, 8, QL], F32, name=f"w{i}") for i in range(K)]
        for kk in range(K):
            nc.vector.tensor_mul(w_t[kk], e_k[kk], rinv)

        # ---- main loop over local captions ----
        for q in range(QL):
            capb_t = qv.tile([B, D], F32, tag="capb")
            nc.sync.dma_start(out=capb_t, in_=capb_d.ap()[q])

            A_ps = psA.tile([B, D], F32, tag="A")
            B_ps = psB.tile([B, D], F32, tag="B")

            for (b0, nb, nch0, nnch) in SLABS:
                xcv = xcp.tile([128, 8, 1152], F32R, tag="xc")
                cols = nb * 36
                for cc in range(8):
                    xo = xcv[:, cc, 0:cols].rearrange("p (b r) -> p b r", r=36)
                    # xcv = x[r+1]*w2 (ScalarE); then two fused MACs (DVE)
                    nc.scalar.mul(xo, x38_t[:, cc, b0:b0 + nb, 2:38],
                                  w_t[2][:, cc, q:q + 1])
                    nc.vector.scalar_tensor_tensor(
                        xo, x38_t[:, cc, b0:b0 + nb, 0:36],
                        w_t[0][:, cc, q:q + 1], xo, OP.mult, OP.add)
                    nc.vector.scalar_tensor_tensor(
                        xo, x38_t[:, cc, b0:b0 + nb, 1:37],
                        w_t[1][:, cc, q:q + 1], xo, OP.mult, OP.add)
                    if nch0 + nnch == NCH:
                        nc.vector.tensor_copy(
                            out=xcv[:, cc, cols:cols + 64], in_=z64)

                for j in range(nnch):
                    nch = nch0 + j
                    y_ps = psy.tile([128, D], F32, tag="y")
                    for h in range(2):
                        for cc in range(8):
                            nc.tensor.matmul(
                                y_ps[:, h * 512:(h + 1) * 512],
                                lhsT=xcv[:, cc, j * 128:(j + 1) * 128],
                                rhs=wct_t[:, cc, h * 512:(h + 1) * 512],
                                start=(cc == 0), stop=(cc == 7))
                    e_t = ep.tile([128, D], F32R, tag="e")
                    for h in range(2):
                        nc.scalar.activation(e_t[:, h * 512:(h + 1) * 512],
                                             y_ps[:, h * 512:(h + 1) * 512],
                                             AF.Exp)
                    p_t = ep.tile([128, D], F32R, tag="p", bufs=1)
                    nc.vector.tensor_mul(p_t, e_t, y_ps)
                    selr = sel_t[:, nch, :]
                    for h in range(2):
                        nc.tensor.matmul(
                            A_ps[:, h * 512:(h + 1) * 512], lhsT=selr,
                            rhs=e_t[:, h * 512:(h + 1) * 512],
                            start=(nch == 0), stop=(nch == NCH - 1))
                        nc.tensor.matmul(
                            B_ps[:, h * 512:(h + 1) * 512], lhsT=selr,
                            rhs=p_t[:, h * 512:(h + 1) * 512],
                            start=(nch == 0), stop=(nch == NCH - 1))

            # epilogue: img_vec = B/A + bconv, cosine vs caption
            A_sb = qv.tile([B, D], F32, tag="asb")
            nc.scalar.copy(A_sb, A_ps)
            rA = qv.tile([B, D], F32, tag="ra")
            scr = qv.tile([B, D], F32, tag="scr")
            # 1/A via exp(-ln(A)) on ScalarE (A > 0); custom-DVE recip
            # is unsupported on this runtime
            nc.scalar.activation(rA, A_sb, AF.Ln)
            nc.scalar.activation(rA, rA, AF.Exp, scale=-1.0)
            nc.vector.tensor_mul(A_sb, bcb_t, A_sb)   # bconv * A
            nc.vector.tensor_add(A_sb, A_sb, B_ps)    # + B
            v_t = A_sb
            nc.vector.tensor_mul(v_t, v_t, rA)        # img_vec
            nc.vector.tensor_mul(scr, v_t, capb_t)
            nc.vector.tensor_reduce(dot_t[:, q:q + 1], scr,
                                    mybir.AxisListType.X, OP.add)
            nc.vector.tensor_mul(scr, v_t, v_t)
            nc.vector.tensor_reduce(s2_t[:, q:q + 1], scr,
                                    mybir.AxisListType.X, OP.add)
            nc.vector.tensor_mul(scr, capb_t, capb_t)
            nc.vector.tensor_reduce(s2c_t[:, q:q + 1], scr,
                                    mybir.AxisListType.X, OP.add)

        # sims = dot / sqrt(s2 * s2c)  via exp(-0.5 ln(.))
        den = small.tile([B, QL], F32)
        nc.vector.tensor_mul(den, s2_t, s2c_t)
        lg = small.tile([B, QL], F32)
        nc.scalar.activation(lg, den, AF.Ln)
        rs = small.tile([B, QL], F32)
        nc.scalar.activation(rs, lg, AF.Exp, scale=-0.5)
        nc.vector.tensor_mul(out_sb, dot_t, rs)
        nc.sync.dma_start(out=out_d.ap(), in_=out_sb)

    nc.compile()
    return nc


def _chunked(a):
    """(D, ...) -> (128, 8, ...) with d = c*128 + p."""
    return np.ascontiguousarray(
        a.reshape(8, 128, *a.shape[1:]).transpose(1, 0, *range(2, a.ndim + 1)))


def _prep_shared(img, Wred, Wproj, Wconv):
    xt = np.ascontiguousarray(img.transpose(2, 0, 1))       # (D, B, R)
    x38 = np.zeros((D, B, 38), np.float32)
    x38[:, :, 1:37] = xt
    x38 = _chunked(x38)                                      # (128,8,B,38)
    wct = _chunked(np.ascontiguousarray(Wconv.T))            # (128,8,D)
    wrt = _chunked(np.ascontiguousarray(Wred.T))             # (128,8,DQ)
    wpp = np.ascontiguousarray(                              # (128,2,K,D)
        Wproj.reshape(D, K, DQ).transpose(2, 1, 0)
        .reshape(2, 128, K, D).transpose(1, 0, 2, 3))
    sel = np.zeros((128, NCH, B), np.float32)
    n = np.arange(NP)
    valid = n < NB
    sel[n[valid] % 128, n[valid] // 128, n[valid] // R] = 1.0
    return x38, wct, wrt, wpp, sel


def kernel(img_embed, cap_embed, lens, Wred, bred, Wproj, bproj, Wconv,
           bconv, **_unused):
    global LAST_EXEC_NS
    img_embed = np.asarray(img_embed, np.float32)
    cap0 = np.asarray(cap_embed, np.float32)[:, 0, :]        # (Q, D)
    Wred = np.asarray(Wred, np.float32)
    bred_a = np.asarray(bred, np.float32)
    Wproj = np.asarray(Wproj, np.float32)
    bproj_a = np.asarray(bproj, np.float32)
    Wconv = np.asarray(Wconv, np.float32)
    bconv_a = np.asarray(bconv, np.float32)

    if "nc" not in _CACHE:
        _CACHE["nc"] = _build_nc()
    nc = _CACHE["nc"]

    x38, wct, wrt, wpp, sel = _prep_shared(img_embed, Wred, Wproj, Wconv)
    bred_s = np.ascontiguousarray(bred_a.reshape(2, 128).T)
    bpp = _chunked(bproj_a.reshape(D, K))                     # (128,8,K)
    bcb = np.ascontiguousarray(np.broadcast_to(bconv_a, (B, D)))

    in_maps = []
    for c in range(N_CORES):
        capq = cap0[c * QL:(c + 1) * QL]                      # (QL, D)
        capt = _chunked(np.ascontiguousarray(capq.T))         # (128,8,QL)
        capb = np.ascontiguousarray(
            np.broadcast_to(capq[:, None, :], (QL, B, D)))
        in_maps.append({
            "x38": x38, "wct": wct, "wrt": wrt, "wpp": wpp,
            "bred": bred_s, "bpp": bpp, "sel": sel, "bcb": bcb,
            "capt": capt, "capb": capb,
        })

    trace = bool(int(os.environ.get("KTRACE", "0")))
    tdir = os.environ.get("KTRACE_DIR") or None
    res = run_bass_kernel_spmd(nc, in_maps, core_ids=list(range(N_CORES)),
                               trace=trace, tmpdir=tdir)
    LAST_EXEC_NS = res.exec_time_ns
    return np.concatenate([res.results[c]["out"] for c in range(N_CORES)],
                          axis=1)



# revision 2
# speedup vs baseline: 1.0439x; 1.0439x over previous
"""Trainium2 Bass kernel for nn_KernelProjectionT2I — v2 (fp8 DoubleRow).

Sharding: data-parallel over captions (B_cap=48 -> 6 per core on 8 cores).

Math (per caption q), with SC = 32 and fixed mean-taps WBAR = 1/3:
  wdyn = softmax_K(MLP(cap0))                      (per-channel 3 taps)
  xcv  = w0.x[r-1] + w1.x[r] + w2.x[r+1]
       = xm + dw0.d0 + dw2.d2,   xm = x + (d0+d2)/3  (host, bf16)
         d0 = x[r-1]-x[r], d2 = x[r+1]-x[r]          (host, bf16)
         dw_k = w_k - 1/3                            (device)
  SC*y = ym + Wconv8 @ du ;  ym = (SC*Wconv_bf16) @ xm  (one-time, bf16)
         du = dw0.d0 + dw2.d2 (fp8), Wconv8 = fp8(SC*Wconv), DoubleRow
  e = exp(y) (bf16), p' = (SC*y).e = SC*p (bf16)
  A = sel @ e, B' = sel @ p'  (TensorE region sums, bf16 matmuls)
  v = (B'/SC)/A + bconv ; sims = <v, cap_unit>/|v|

ym is injected into each chunk's PSUM before the DoubleRow accumulation:
ACT copy for some chunks, identity-matmul for the rest.
"""

import numpy as np
import ml_dtypes
from contextlib import ExitStack

import concourse.bass as bass
import concourse.tile as tile
from concourse import bacc, mybir
from concourse.bass_utils import run_bass_kernel_spmd

F32 = mybir.dt.float32
F32R = mybir.dt.float32r
BF16 = mybir.dt.bfloat16
F8 = mybir.dt.float8e4
AF = mybir.ActivationFunctionType
OP = mybir.AluOpType
DR = mybir.MatmulPerfMode.DoubleRow
AXX = mybir.AxisListType.X

N_CORES = 8
B, R, D = 48, 36, 1024
Q = 48
QL = Q // N_CORES
DQ, K = 256, 3
NB = B * R                 # 1728
NP = 1792                  # 14 chunks of 128
NCH = NP // 128
SC = 32.0
WBAR = 1.0 / 3.0

ACT_INJ = {0, 2, 4, 6, 8, 10, 12, 13}  # ym-inject via ACT; rest PE identity


def _dedup_ldweights(m):
    """Drop back-to-back Ldweights that reload the identical stationary.

    bass' move_matmul_waits_to_ldweights emits one Ldweights per matmul;
    consecutive matmuls sharing lhsT reload the PE array needlessly. Safe
    to delete when the duplicate carries no semaphore waits/updates and no
    other PE-array-mutating instruction sits in between.
    """
    removed = 0
    for f in m.functions:
        for blk in f.blocks:
            last_key = None
            keep = []
            for inst in blk.instructions:
                if inst.opcode == "Ldweights":
                    key = (str(inst.ins[0]), getattr(inst, "perf_mode", None))
                    if (key == last_key and not inst.has_wait()
                            and not inst.has_update()):
                        removed += 1
                        continue
                    last_key = key
                elif str(getattr(inst, "engine", None)) == "EngineType.PE" \
                        and inst.opcode not in ("Matmult", "EventSemaphore",
                                                "Drain"):
                    last_key = None
                keep.append(inst)
            blk.instructions[:] = keep
    return removed

LAST_EXEC_NS = None
_CACHE = {}
import os


def _build_nc():
    nc = bacc.Bacc(trn_type="TRN2", target_bir_lowering=False,
                   num_devices=N_CORES)
    capt_d = nc.dram_tensor("capt", [128, 8, QL], F32, kind="ExternalInput")
    wrt_d = nc.dram_tensor("wrt", [128, 8, DQ], F32, kind="ExternalInput")
    wpp_d = nc.dram_tensor("wpp", [128, 2, K, D], F32, kind="ExternalInput")
    bred_d = nc.dram_tensor("bred", [128, 2], F32, kind="ExternalInput")
    bpp_d = nc.dram_tensor("bpp", [128, 8, K], F32, kind="ExternalInput")
    d0_d = nc.dram_tensor("d0b", [128, 8, NB], BF16, kind="ExternalInput")
    d2_d = nc.dram_tensor("d2b", [128, 8, NB], BF16, kind="ExternalInput")
    xm_d = nc.dram_tensor("xmb", [128, 8, NP], BF16, kind="ExternalInput")
    wct8_d = nc.dram_tensor("wct8", [128, 8, D], F8, kind="ExternalInput")
    wctb_d = nc.dram_tensor("wctb", [128, 8, D], BF16, kind="ExternalInput")
    selb_d = nc.dram_tensor("selb", [128, NCH, B], BF16, kind="ExternalInput")
    idb_d = nc.dram_tensor("idb", [128, 128], BF16, kind="ExternalInput")
    capu_d = nc.dram_tensor("capu", [QL, B, D], BF16, kind="ExternalInput")
    bcb_d = nc.dram_tensor("bcb", [B, D], BF16, kind="ExternalInput")
    out_d = nc.dram_tensor("out", [B, QL], F32, kind="ExternalOutput")

    with ExitStack() as ctx:
        tc = ctx.enter_context(tile.TileContext(nc))
        const = ctx.enter_context(tc.tile_pool(name="const", bufs=1))
        big = ctx.enter_context(tc.tile_pool(name="big", bufs=2))
        tp = ctx.enter_context(tc.tile_pool(name="tp", bufs=2))
        ep = ctx.enter_context(tc.tile_pool(name="ep", bufs=2))
        small = ctx.enter_context(tc.tile_pool(name="small", bufs=2))
        psy = ctx.enter_context(tc.tile_pool(name="psy", bufs=2, space="PSUM"))
        psA = ctx.enter_context(tc.tile_pool(name="psA", bufs=1, space="PSUM"))
        psB = ctx.enter_context(tc.tile_pool(name="psB", bufs=1, space="PSUM"))

        # ---- resident inputs ----
        capt_t = const.tile([128, 8, QL], F32)
        nc.sync.dma_start(out=capt_t, in_=capt_d.ap())
        bred_t = const.tile([128, 2], F32)
        nc.sync.dma_start(out=bred_t, in_=bred_d.ap())
        bpp_t = const.tile([128, 8, K], F32)
        nc.sync.dma_start(out=bpp_t, in_=bpp_d.ap())
        wrt_t = const.tile([128, 8, DQ], F32)
        nc.sync.dma_start(out=wrt_t, in_=wrt_d.ap())
        wpp_t = [tp.tile([128, K, D], F32, tag="tmp", name=f"wpp{m}")
                 for m in range(2)]
        for m in range(2):
            nc.sync.dma_start(out=wpp_t[m], in_=wpp_d.ap()[:, m])
        ym_t = const.tile([128, NCH, D], BF16)
        dot_t = const.tile([B, QL], F32)
        s2_t = const.tile([B, QL], F32)

        # big-pool staging: wctb + xm share the du slots, used once up front
        wctb_t = big.tile([128, 8, D], BF16, tag="du", name="wctb")
        nc.sync.dma_start(out=wctb_t, in_=wctb_d.ap())
        xm_t = big.tile([128, 8, NP], BF16, tag="du", name="xm")
        nc.sync.dma_start(out=xm_t, in_=xm_d.ap())
        d0_t = const.tile([128, 8, NB], BF16)
        nc.sync.dma_start(out=d0_t, in_=d0_d.ap())
        d2_t = const.tile([128, 8, NB], BF16)
        nc.sync.dma_start(out=d2_t, in_=d2_d.ap())
        wct8_t = const.tile([128, 8, D], F8)
        nc.sync.dma_start(out=wct8_t, in_=wct8_d.ap())
        selb_t = const.tile([128, NCH, B], BF16)
        nc.sync.dma_start(out=selb_t, in_=selb_d.ap())
        idb_t = const.tile([128, 128], BF16)
        nc.sync.dma_start(out=idb_t, in_=idb_d.ap())
        bcb_t = const.tile([B, D], BF16)
        nc.sync.dma_start(out=bcb_t, in_=bcb_d.ap())

        # ---- caption MLP for local captions (fp32) ----
        repr_ps = psB.tile([128, 2, QL], F32, tag="B")
        for mc in range(2):
            for cc in range(8):
                nc.tensor.matmul(repr_ps[:, mc, :],
                                 lhsT=wrt_t[:, cc, mc * 128:(mc + 1) * 128],
                                 rhs=capt_t[:, cc, :],
                                 start=(cc == 0), stop=(cc == 7))
        repr_sb = small.tile([128, 2, QL], F32)
        for mc in range(2):
            nc.vector.tensor_scalar_add(repr_sb[:, mc, :], repr_ps[:, mc, :],
                                        bred_t[:, mc:mc + 1])

        L_ps = [psy.tile([128, 8, QL], F32, tag="y", name="L0"),
                psy.tile([128, 8, QL], F32, tag="y", name="L1"),
                psA.tile([128, 8, QL], F32, tag="A", name="L2")]
        for kk in range(K):
            for mc in range(8):
                nc.tensor.matmul(L_ps[kk][:, mc, :],
                                 lhsT=wpp_t[0][:, kk, mc * 128:(mc + 1) * 128],
                                 rhs=repr_sb[:, 0, :], start=True, stop=False)
                nc.tensor.matmul(L_ps[kk][:, mc, :],
                                 lhsT=wpp_t[1][:, kk, mc * 128:(mc + 1) * 128],
                                 rhs=repr_sb[:, 1, :], start=False, stop=True)

        # softmax over K taps (no max-sub: |logits| ~ N(0,1))
        e_k = [small.tile([128, 8, QL], F32, name=f"ek{i}") for i in range(K)]
        for kk in range(K):
            for mc in range(8):
                nc.scalar.activation(e_k[kk][:, mc, :], L_ps[kk][:, mc, :],
                                     AF.Exp, bias=bpp_t[:, mc, kk:kk + 1])
        ssum = small.tile([128, 8, QL], F32)
        nc.vector.tensor_add(ssum, e_k[0], e_k[1])
        nc.vector.tensor_add(ssum, ssum, e_k[2])
        rinv = small.tile([128, 8, QL], F32)
        nc.vector.reciprocal(rinv, ssum)
        # dw_k = softmax_k - 1/3 for taps 0 and 2
        dw_t = [const.tile([128, 8, QL], F32, name=f"dw{i}") for i in (0, 2)]
        for i, kk in enumerate((0, 2)):
            nc.vector.tensor_mul(dw_t[i], e_k[kk], rinv)
            nc.vector.tensor_scalar_sub(dw_t[i], dw_t[i], WBAR)

        # ---- one-time: ym = (SC*Wconv_bf) @ xm, stored bf16 in SBUF ----
        for j in range(NCH):
            ym_ps = psy.tile([128, D], F32, tag="y")
            for cc in range(8):
                for h in range(2):
                    nc.tensor.matmul(
                        ym_ps[:, h * 512:(h + 1) * 512],
                        lhsT=xm_t[:, cc, j * 128:(j + 1) * 128],
                        rhs=wctb_t[:, cc, h * 512:(h + 1) * 512],
                        start=(cc == 0), stop=(cc == 7))
            nc.scalar.copy(ym_t[:, j, :], ym_ps)

        # ---- main loop over local captions ----
        for q in range(QL):
            capuq = small.tile([B, D], BF16, tag="capq", bufs=1)
            nc.sync.dma_start(out=capuq, in_=capu_d.ap()[q])
            du = big.tile([128, 8, NP], F8, tag="du")
            nc.gpsimd.memset(du[:, :, NB:NP], 0.0)
            for cc in range(8):
                tmp = tp.tile([128, NB], BF16, tag="tmp")
                nc.vector.tensor_scalar_mul(tmp, d0_t[:, cc],
                                            dw_t[0][:, cc, q:q + 1])
                nc.vector.scalar_tensor_tensor(du[:, cc, 0:NB], d2_t[:, cc],
                                               dw_t[1][:, cc, q:q + 1], tmp,
                                               OP.mult, OP.add)

            A_ps = psA.tile([B, D], F32, tag="A")
            B_ps = psB.tile([B, D], F32, tag="B")
            for j in range(NCH):
                y_ps = psy.tile([128, D], F32, tag="y")
                if j in ACT_INJ:
                    nc.scalar.copy(y_ps, ym_t[:, j, :])
                else:
                    for h in range(2):
                        nc.tensor.matmul(y_ps[:, h * 512:(h + 1) * 512],
                                         lhsT=idb_t,
                                         rhs=ym_t[:, j, h * 512:(h + 1) * 512],
                                         start=True, stop=False)
                for t in range(4):
                    for h in range(2):
                        nc.tensor.matmul(
                            y_ps[:, h * 512:(h + 1) * 512],
                            lhsT=du[:, 2 * t:2 * t + 2, j * 128:(j + 1) * 128],
                            rhs=wct8_t[:, 2 * t:2 * t + 2,
                                       h * 512:(h + 1) * 512],
                            perf_mode=DR, start=False, stop=(t == 3),
                            skip_group_check=True)
                e_t = ep.tile([128, D], BF16, tag="e")
                nc.scalar.activation(e_t, y_ps, AF.Exp, scale=1.0 / SC)
                p_t = ep.tile([128, D], BF16, tag="p")
                nc.vector.tensor_mul(p_t, y_ps, e_t)
                selr = selb_t[:, j, :]
                for h in range(2):
                    nc.tensor.matmul(A_ps[:, h * 512:(h + 1) * 512],
                                     lhsT=selr,
                                     rhs=e_t[:, h * 512:(h + 1) * 512],
                                     start=(j == 0), stop=(j == NCH - 1))
                    nc.tensor.matmul(B_ps[:, h * 512:(h + 1) * 512],
                                     lhsT=selr,
                                     rhs=p_t[:, h * 512:(h + 1) * 512],
                                     start=(j == 0), stop=(j == NCH - 1))

            # epilogue: v = (B'/SC)/A + bconv; dot/s2 via ACT accumulators
            Bsb = small.tile([B, D], BF16, tag="bsb", bufs=1)
            nc.scalar.activation(Bsb, B_ps, AF.Copy, scale=1.0 / SC)
            # rA = 1/A via exp(-ln(A)) on ACT (A > 0)
            lnA = small.tile([B, D], F32, tag="lna", bufs=1)
            nc.scalar.activation(lnA, A_ps, AF.Ln)
            rA = small.tile([B, D], BF16, tag="ra", bufs=1)
            nc.scalar.activation(rA, lnA, AF.Exp, scale=-1.0)
            v_t = small.tile([B, D], BF16, tag="v", bufs=1)
            nc.gpsimd.tensor_mul(v_t, Bsb, rA)
            nc.gpsimd.tensor_add(v_t, v_t, bcb_t)
            scr = small.tile([B, D], BF16, tag="scr", bufs=1)
            nc.gpsimd.tensor_mul(scr, v_t, capuq)
            nc.scalar.activation(scr, scr, AF.Copy,
                                 accum_out=dot_t[:, q:q + 1])
            nc.scalar.activation(scr, v_t, AF.Square,
                                 accum_out=s2_t[:, q:q + 1])

        # sims = dot * sqrt(1/s2)
        rs2 = small.tile([B, QL], F32, name="rs2")
        nc.vector.reciprocal(rs2, s2_t)
        rrt = small.tile([B, QL], F32, name="rrt")
        nc.scalar.activation(rrt, rs2, AF.Sqrt)
        out_sb = small.tile([B, QL], F32, name="osb")
        nc.vector.tensor_mul(out_sb, dot_t, rrt)
        nc.sync.dma_start(out=out_d.ap(), in_=out_sb)

    nc.compile()
    _dedup_ldweights(nc.m)
    return nc


def _chunked(a):
    """(D, ...) -> (128, 8, ...) with d = cc*128 + p."""
    return np.ascontiguousarray(
        a.reshape(8, 128, *a.shape[1:]).transpose(1, 0, *range(2, a.ndim + 1)))


def kernel(img_embed, cap_embed, lens, Wred, bred, Wproj, bproj, Wconv,
           bconv, **_unused):
    global LAST_EXEC_NS
    f8 = ml_dtypes.float8_e4m3
    bf = ml_dtypes.bfloat16
    img_embed = np.asarray(img_embed, np.float32)
    cap0 = np.asarray(cap_embed, np.float32)[:, 0, :]        # (Q, D)
    Wred_a = np.asarray(Wred, np.float32)
    bred_a = np.asarray(bred, np.float32)
    Wproj_a = np.asarray(Wproj, np.float32)
    bproj_a = np.asarray(bproj, np.float32)
    Wconv_a = np.asarray(Wconv, np.float32)
    bconv_a = np.asarray(bconv, np.float32)

    if "nc" not in _CACHE:
        _CACHE["nc"] = _build_nc()
    nc = _CACHE["nc"]

    x = np.ascontiguousarray(img_embed.transpose(2, 0, 1))   # (D, B, R)
    xpad = np.pad(x, ((0, 0), (0, 0), (1, 1)))
    d0 = xpad[:, :, 0:R] - x
    d2 = xpad[:, :, 2:R + 2] - x
    xm = x + WBAR * (d0 + d2)
    d0b = _chunked(d0.reshape(D, NB)).astype(bf)
    d2b = _chunked(d2.reshape(D, NB)).astype(bf)
    xmb = np.zeros((128, 8, NP), bf)
    xmb[:, :, 0:NB] = _chunked(xm.reshape(D, NB)).astype(bf)
    wct8 = _chunked(np.ascontiguousarray(Wconv_a.T * SC)).astype(f8)
    wctb = _chunked(np.ascontiguousarray(Wconv_a.T * SC)).astype(bf)
    wrt = _chunked(np.ascontiguousarray(Wred_a.T))
    wpp = np.ascontiguousarray(
        Wproj_a.reshape(D, K, DQ).transpose(2, 1, 0)
        .reshape(2, 128, K, D).transpose(1, 0, 2, 3))
    bred_s = np.ascontiguousarray(bred_a.reshape(2, 128).T)
    bpp = _chunked(bproj_a.reshape(D, K))
    selb = np.zeros((128, NCH, B), bf)
    n = np.arange(NP)
    valid = n < NB
    selb[n[valid] % 128, n[valid] // 128, n[valid] // R] = 1.0
    idb = np.eye(128, dtype=bf)
    bcb = np.ascontiguousarray(np.broadcast_to(bconv_a, (B, D))).astype(bf)
    capn = cap0 / np.linalg.norm(cap0, axis=1, keepdims=True)

    in_maps = []
    for c in range(N_CORES):
        capq = cap0[c * QL:(c + 1) * QL]                      # (QL, D)
        capt = _chunked(np.ascontiguousarray(capq.T))         # (128,8,QL)
        capu = np.ascontiguousarray(np.broadcast_to(
            capn[c * QL:(c + 1) * QL][:, None, :], (QL, B, D))).astype(bf)
        in_maps.append({
            "capt": capt, "wrt": wrt, "wpp": wpp, "bred": bred_s, "bpp": bpp,
            "d0b": d0b, "d2b": d2b, "xmb": xmb, "wct8": wct8, "wctb": wctb,
            "selb": selb, "idb": idb, "capu": capu, "bcb": bcb,
        })

    trace = bool(int(os.environ.get("KTRACE", "0")))
    tdir = os.environ.get("KTRACE_DIR") or None
    res = run_bass_kernel_spmd(nc, in_maps, core_ids=list(range(N_CORES)),
                               trace=trace, tmpdir=tdir)
    LAST_EXEC_NS = res.exec_time_ns
    return np.concatenate([res.results[c]["out"].astype(np.float32)
                           for c in range(N_CORES)], axis=1)
